# revision 27
# baseline (speedup 1.0000x reference)
"""Causal multi-head attention with RoPE on 8 TRN2 NeuronCores.

Problem: BS=2, SEQ=2048, DIM=2048, NH=16, HD=128 (fp32 in/out).
Sharding: core = b*4 + g  (b = batch, g = head-group of 4 heads).
Each core computes q/k/v for its 4 heads from its batch's x, applies RoPE,
causal attention, and a partial output projection through its 512-row slice
of wo. The host sums the 4 per-group partials per batch.

v4 (bf16 operands):
  Phase V first (sb-major, needs only 1/4 of x to start) while the rest of
  x and the K/Q weights stream in; then K+Q m-major (weight tile fixed,
  4 seq-blocks inner -> lhsT reuse). Q is scaled by 1/sqrt(HD) during the
  PSUM->SBUF copy so K and Q share one trig table. x stays resident in
  SBUF across both passes. Attention processes score tiles in PAIRS
  sharing a 2-bank PSUM tile so one ACT exp covers 1024 columns; diagonal
  blocks trimmed to (512,384,256,128) columns. Rowsum via ones-matmul.
  1/rowsum = ACT Ln + Exp(-x), emitted lazily after the next block's first
  pair so table swaps stay off the PSUM-reuse critical path. wo pass pairs
  dblk matmuls per lhsT and interleaves with the next block's attention.
"""
import math
import numpy as np
from contextlib import ExitStack

import concourse.bass as bass
import concourse.bacc as bacc
import concourse.tile as tile
import concourse.mybir as mybir
from concourse import bass_utils

F32 = mybir.dt.float32
F32R = mybir.dt.float32r
BF16 = mybir.dt.bfloat16
AF = mybir.ActivationFunctionType

SEQ = 2048
DIM = 2048
HD = 128
MG = 512                       # per-core head width (4 heads x 128)
ND = DIM // 128                # 16 d-tiles
PAIR_SWAP = [i ^ 1 for i in range(32)]

MM_DT = BF16                   # matmul operand dtype

_CACHED = {}


def build_nc(seq=SEQ, mm_dt=None):
    mm_dt = mm_dt or MM_DT
    NSB = seq // 512
    NST = seq // 128
    qscale = float(1.0 / math.sqrt(HD))
    nc = bacc.Bacc("TRN2", target_bir_lowering=False, debug=False)

    x_d = nc.dram_tensor("x_t", [NSB, 128, ND, 512], mm_dt, kind="ExternalInput")
    wq_d = nc.dram_tensor("wq_t", [128, ND, 512], mm_dt, kind="ExternalInput")
    wk_d = nc.dram_tensor("wk_t", [128, ND, 512], mm_dt, kind="ExternalInput")
    wv_d = nc.dram_tensor("wv_t", [128, ND, 512], mm_dt, kind="ExternalInput")
    wo_d = nc.dram_tensor("wo_t", [128, 4, DIM], mm_dt, kind="ExternalInput")
    trk_d = nc.dram_tensor("trigk", [128, 2, seq], F32, kind="ExternalInput")
    msk_d = nc.dram_tensor("masks_t", [128, 4, 512], BF16, kind="ExternalInput")
    onc_d = nc.dram_tensor("ones_sq", [128, 128], mm_dt, kind="ExternalInput")
    out_d = nc.dram_tensor("out", [seq, DIM], F32, kind="ExternalOutput")

    with tile.TileContext(nc) as tc, ExitStack() as ctx:
        persist = ctx.enter_context(tc.tile_pool(name="persist", bufs=1))
        ktr = [persist.tile([128, seq], mm_dt, tag=f"ktr{h}", name=f"ktr{h}")
               for h in range(4)]
        v_sb = persist.tile([128, NST, MG], mm_dt, tag="v")
        ones_sq = persist.tile([128, 128], mm_dt, tag="onesq")
        msk = persist.tile([128, 4, 512], BF16, tag="masks")
        qtrp = ctx.enter_context(tc.tile_pool(name="qtrp", bufs=1))
        qtr = [qtrp.tile([128, seq], mm_dt, tag=f"qtr{h}", name=f"qtr{h}")
               for h in range(4)]

        # trig + rope scratch: DVE-consumed; keep open past phase A so the
        # pre-attention pool drain only waits on PE-consumed tiles
        trigp = ctx.enter_context(tc.tile_pool(name="trigp", bufs=1))
        trigk_sb = trigp.tile([128, 2, seq], F32, tag="trigk")
        prawp = ctx.enter_context(tc.tile_pool(name="prawp", bufs=3))
        ropet = ctx.enter_context(tc.tile_pool(name="ropep", bufs=2))

        # long-lived within phases V+A only: x and the K/Q weights (PE-read)
        s1 = ExitStack()
        xpool = s1.enter_context(tc.tile_pool(name="xp", bufs=1))
        xall = xpool.tile([128, NSB, ND, 512], mm_dt, tag="xall")
        paw = s1.enter_context(tc.tile_pool(name="paw", bufs=1))
        wk_sb = paw.tile([128, ND, MG], mm_dt, tag="wk")
        wq_sb = paw.tile([128, ND, MG], mm_dt, tag="wq")

        def rope(psum_t, out_slice, trig_t, scale=1.0):
            """out = p*cos + shuffle(p)*sin; ACT-copy psum->sbuf first so the
            PSUM bank frees without waiting the DVE chain."""
            praw = prawp.tile([128, 512], F32, tag="praw", name="praw")
            if scale == 1.0:
                nc.scalar.copy(praw[:], psum_t[:])
            else:
                nc.scalar.activation(praw[:], psum_t[:], AF.Copy, scale=scale)
            shuf = ropet.tile([128, 512], F32, tag="shuf", name="shuf")
            nc.vector.stream_shuffle(shuf[:], praw[:], PAIR_SWAP)
            t1 = ropet.tile([128, 512], F32, tag="t1", name="t1")
            nc.vector.tensor_mul(t1[:], praw[:], trig_t[:, 0, :])
            nc.vector.tensor_mul(shuf[:], shuf[:], trig_t[:, 1, :])
            nc.vector.tensor_add(out_slice, t1[:], shuf[:])

        def load_w_quarters(dst, dram_ap):
            for q4 in range(4):
                nc.sync.dma_start(dst[:, q4 * 4:(q4 + 1) * 4, :],
                                  dram_ap[:, q4 * 4:(q4 + 1) * 4, :])

        # ---- phase V: V = x^T @ wv, sb-major; x/weights stream in ----
        with tc.tile_pool(name="pbw", bufs=1) as pbw, \
             tc.tile_pool(name="psv", bufs=1, space="PSUM") as psv:
            wv_w = pbw.tile([128, ND, MG], mm_dt, tag="wvw")
            # first V matmul needs only wv quarter 0 + x[sb0, dt0-1]:
            # issue those triggers first so PE starts ~4us in
            nc.sync.dma_start(wv_w[:, 0:4, :], wv_d.ap()[:, 0:4, :])
            nc.sync.dma_start(xall[:, 0, 0:2, :], x_d.ap()[0, :, 0:2, :])
            nc.sync.dma_start(xall[:, 0, 2:8, :], x_d.ap()[0, :, 2:8, :])
            for q4 in range(1, 4):
                nc.sync.dma_start(wv_w[:, q4 * 4:(q4 + 1) * 4, :],
                                  wv_d.ap()[:, q4 * 4:(q4 + 1) * 4, :])
            nc.sync.dma_start(xall[:, 0, 8:16, :], x_d.ap()[0, :, 8:16, :])
            nc.sync.dma_start(ones_sq[:], onc_d.ap())
            nc.sync.dma_start(msk[:], msk_d.ap())
            for sb in range(1, NSB):
                nc.sync.dma_start(xall[:, sb, 0:8, :], x_d.ap()[sb, :, 0:8, :])
                nc.sync.dma_start(xall[:, sb, 8:16, :],
                                  x_d.ap()[sb, :, 8:16, :])
            load_w_quarters(wk_sb, wk_d.ap())
            load_w_quarters(wq_sb, wq_d.ap())
            nc.sync.dma_start(trigk_sb[:], trk_d.ap())
            for sb in range(NSB):
                ps_v = [psv.tile([128, 512], F32, tag=f"psv{st}",
                                 name=f"psv{st}") for st in range(4)]
                for dt in range(ND):
                    for st in range(4):
                        nc.tensor.matmul(
                            ps_v[st][:],
                            xall[:, sb, dt, st * 128:(st + 1) * 128],
                            wv_w[:, dt, :],
                            start=(dt == 0), stop=(dt == ND - 1))
                for st in range(4):
                    nc.scalar.copy(v_sb[:, sb * 4 + st, :], ps_v[st][:])

        # ---- phase A: K + Q, m-major (lhsT weight reused across 4 sb) ----
        with tc.tile_pool(name="psk", bufs=1, space="PSUM") as psk, \
             tc.tile_pool(name="psq", bufs=1, space="PSUM") as psq:
            for g in range(4):
                ps_k = [psk.tile([128, 512], F32, tag=f"psk{sb}",
                                 name=f"psk{sb}") for sb in range(NSB)]
                for dt in range(ND):
                    for sb in range(NSB):
                        nc.tensor.matmul(
                            ps_k[sb][:],
                            wk_sb[:, dt, g * 128:(g + 1) * 128],
                            xall[:, sb, dt, :],
                            start=(dt == 0), stop=(dt == ND - 1))
                for sb in range(NSB):
                    rope(ps_k[sb], ktr[g][:, sb * 512:(sb + 1) * 512],
                         trigk_sb[:, :, sb * 512:(sb + 1) * 512])
                ps_q = [psq.tile([128, 512], F32, tag=f"psq{sb}",
                                 name=f"psq{sb}") for sb in range(NSB)]
                if g < 3:
                    for dt in range(ND):
                        for sb in range(NSB):
                            nc.tensor.matmul(
                                ps_q[sb][:],
                                wq_sb[:, dt, g * 128:(g + 1) * 128],
                                xall[:, sb, dt, :],
                                start=(dt == 0), stop=(dt == ND - 1))
                    for sb in range(NSB):
                        rope(ps_q[sb], qtr[g][:, sb * 512:(sb + 1) * 512],
                             trigk_sb[:, :, sb * 512:(sb + 1) * 512],
                             scale=qscale)
                else:
                    # last group sb-outer: each seq-block's rope fires as its
                    # accumulation stops, so the final rope's DVE chain does
                    # not trail the last matmul by the whole group
                    for sb in range(NSB):
                        for dt in range(ND):
                            nc.tensor.matmul(
                                ps_q[sb][:],
                                wq_sb[:, dt, g * 128:(g + 1) * 128],
                                xall[:, sb, dt, :],
                                start=(dt == 0), stop=(dt == ND - 1))
                        rope(ps_q[sb], qtr[g][:, sb * 512:(sb + 1) * 512],
                             trigk_sb[:, :, sb * 512:(sb + 1) * 512],
                             scale=qscale)

        s1.close()

        # ---- phase 2: attention + wo ----
        with tc.tile_pool(name="p2w", bufs=1) as p2wp, \
             tc.tile_pool(name="ep", bufs=3) as ep, \
             tc.tile_pool(name="etmp", bufs=2) as etmp, \
             tc.tile_pool(name="otn", bufs=2) as otn, \
             tc.tile_pool(name="bcp", bufs=2) as bcp, \
             tc.tile_pool(name="wout", bufs=3) as wout, \
             tc.tile_pool(name="ps_s", bufs=2, space="PSUM") as ps_s, \
             tc.tile_pool(name="ps_o", bufs=1, space="PSUM") as ps_o, \
             tc.tile_pool(name="ps_r", bufs=1, space="PSUM") as ps_r, \
             tc.tile_pool(name="ps_w", bufs=1, space="PSUM") as ps_w:
            wo_sb = p2wp.tile([128, 4, DIM], mm_dt, tag="wo")
            for c in range(4):
                nc.sync.dma_start(wo_sb[:, c, :], wo_d.ap()[:, c, :])

            wo_ops = []          # pending wo-projection micro-ops (thunks)

            def pop_wo_ops(n):
                while n > 0 and wo_ops:
                    wo_ops.pop(0)()
                    n -= 1

            def queue_wo_block(outn_t, ibp, it, tail=False):
                """Queue one [128-row x 2048] slice of the wo projection as
                thunks: 2 halves x (bank alloc + 8 matmuls + 2 copy/dma)."""
                i0p = ibp * 512
                state = {}
                for half in range(2):
                    def alloc(half=half):
                        state[half] = [ps_w.tile([128, 512], F32, tag=f"w{dh}",
                                                 name=f"w{dh}")
                                       for dh in range(2)]
                    wo_ops.append(alloc)
                    for c in range(4):
                        for dh in range(2):
                            def mm(half=half, c=c, dh=dh):
                                dblk = half * 2 + dh
                                nc.tensor.matmul(
                                    state[half][dh][:],
                                    outn_t[:, c, it * 128:(it + 1) * 128],
                                    wo_sb[:, c, dblk * 512:(dblk + 1) * 512],
                                    start=(c == 0), stop=(c == 3))
                            wo_ops.append(mm)

                    def cpdma(half=half):
                        for dh in range(2):
                            dblk = half * 2 + dh
                            ow = wout.tile([128, 512], F32, tag="ow",
                                           name="ow")
                            if tail:
                                nc.scalar.copy(ow[:], state[half][dh][:])
                            else:
                                nc.vector.tensor_copy(ow[:], state[half][dh][:])
                            nc.sync.dma_start(
                                out_d.ap()[i0p + it * 128:
                                           i0p + (it + 1) * 128,
                                           dblk * 512:(dblk + 1) * 512],
                                ow[:])
                    wo_ops.append(cpdma)

            for ib in range(NSB):
                i0 = ib * 512
                nj = 4 * ib + 4
                n_pairs = (nj // 2) * 4
                per_pair = -(-len(wo_ops) // n_pairs) if wo_ops else 0
                outn = otn.tile([128, 4, 512], mm_dt, tag="outn", name="outn")
                for h in range(4):
                    po = ps_o.tile([128, 512], F32, tag="pv", name="pv")
                    prbc = ps_r.tile([128, 512], F32, tag="rs", name="rs")
                    for tp in range(nj // 2):
                        specs = []
                        for q in range(2):
                            tj = 2 * tp + q
                            r = tj - 4 * ib
                            i_lo = 128 * r if r > 0 else 0
                            specs.append((tj, r, i_lo, 512 - i_lo))
                        pscr = ps_s.tile([128, 2, 512], F32, tag="sc", name="sc")
                        for q, (tj, r, i_lo, nw) in enumerate(specs):
                            nc.tensor.matmul(
                                pscr[:, q, 0:nw],
                                ktr[h][:, tj * 128:(tj + 1) * 128],
                                qtr[h][:, i0 + i_lo:i0 + i_lo + nw],
                                start=True, stop=True)
                        e2 = ep.tile([128, 2, 512], mm_dt, tag="e", name="e")
                        if specs[0][1] >= 0 or specs[1][1] >= 0:
                            for q, (tj, r, i_lo, nw) in enumerate(specs):
                                et = etmp.tile([128, 512], mm_dt, tag="etmp",
                                               name="et")
                                nc.scalar.activation(et[:, 0:nw],
                                                     pscr[:, q, 0:nw], AF.Exp)
                                nc.vector.tensor_mul(e2[:, q, 0:nw],
                                                     et[:, 0:nw],
                                                     msk[:, r, i_lo:512])
                        else:
                            nc.scalar.activation(e2[:, 0:2, :],
                                                 pscr[:, 0:2, :], AF.Exp)
                        for q, (tj, r, i_lo, nw) in enumerate(specs):
                            nc.tensor.matmul(
                                po[:, i_lo:i_lo + nw],
                                v_sb[:, tj, h * 128:(h + 1) * 128],
                                e2[:, q, 0:nw],
                                start=(tj == 0), stop=(tj == nj - 1))
                            nc.tensor.matmul(
                                prbc[:, i_lo:i_lo + nw], ones_sq[:],
                                e2[:, q, 0:nw],
                                start=(tj == 0), stop=(tj == nj - 1))
                        pop_wo_ops(per_pair)
                    nc.vector.tensor_copy(outn[:, h, :], po[:])
                    # 1/rowsum on DVE straight from PSUM (no ACT table swaps)
                    rbc = bcp.tile([128, 512], F32, tag="rbc", name="rbc")
                    nc.vector.reciprocal_approx_fast(rbc[:], prbc[:])
                    rbh = bcp.tile([128, 512], mm_dt, tag="rbh", name="rbh")
                    nc.vector.tensor_copy(rbh[:], rbc[:])
                    nc.vector.tensor_mul(outn[:, h, :], outn[:, h, :], rbh[:])
                pop_wo_ops(len(wo_ops))      # drain any leftover from prev ib
                for it in range(4):
                    queue_wo_block(outn, ib, it, tail=(ib == NSB - 1))

            pop_wo_ops(len(wo_ops))

    nc.compile()
    return nc


def _host_prep(x, freqs_cos, freqs_sin, wq, wk, wv, wo, mm_dt=None, seq=SEQ):
    """Build the 8 per-core input maps with pre-tiled layouts."""
    mm_dt = mm_dt or MM_DT
    npdt = mybir.dt.np(mm_dt)
    bs = x.shape[0]
    NSB = seq // 512

    cos_e = np.repeat(np.asarray(freqs_cos).T, 2, axis=0).astype(np.float32)
    sin_raw = np.repeat(np.asarray(freqs_sin).T, 2, axis=0).astype(np.float32)
    sin_e = sin_raw.copy()
    sin_e[0::2] = -sin_raw[0::2]      # out[2i] = q[2i]cos - q[2i+1]sin
    trigk = np.ascontiguousarray(np.stack([cos_e, sin_e], axis=1))

    jr = np.arange(128)[:, None]
    ir = np.arange(512)[None, :]
    masks = np.zeros((4, 128, 512), dtype=np.float32)
    for r in range(4):
        masks[r] = (128 * r + jr <= ir).astype(np.float32)
    masks_t = np.ascontiguousarray(masks.transpose(1, 0, 2)).astype(mybir.dt.np(BF16))

    ones_sq = np.ones((128, 128), npdt)

    def wtile(w):  # [DIM, 512] -> [128, 16, 512]
        return np.ascontiguousarray(
            np.asarray(w).reshape(ND, 128, MG).transpose(1, 0, 2)).astype(npdt)

    x_t = []
    for b in range(bs):
        xt = np.asarray(x[b]).reshape(NSB, 512, ND, 128).transpose(0, 3, 2, 1)
        x_t.append(np.ascontiguousarray(xt).astype(npdt))

    in_maps = []
    for core in range(8):
        b, g = divmod(core, 4)
        b = min(b, bs - 1)
        wo_g = np.asarray(wo)[g * MG:(g + 1) * MG, :]
        in_maps.append({
            "x_t": x_t[b],
            "wq_t": wtile(np.asarray(wq)[:, g * MG:(g + 1) * MG]),
            "wk_t": wtile(np.asarray(wk)[:, g * MG:(g + 1) * MG]),
            "wv_t": wtile(np.asarray(wv)[:, g * MG:(g + 1) * MG]),
            "wo_t": np.ascontiguousarray(
                wo_g.reshape(4, 128, DIM).transpose(1, 0, 2)).astype(npdt),
            "trigk": trigk,
            "masks_t": masks_t, "ones_sq": ones_sq,
        })
    return in_maps


def kernel(x, freqs_cos, freqs_sin, mask, wq, wk, wv, wo, _trace=False):
    x = np.asarray(x, dtype=np.float32)
    in_maps = _host_prep(x, np.asarray(freqs_cos), np.asarray(freqs_sin),
                         np.asarray(wq), np.asarray(wk), np.asarray(wv),
                         np.asarray(wo))
    if "nc" not in _CACHED:
        _CACHED["nc"] = build_nc()
    nc = _CACHED["nc"]
    res = bass_utils.run_bass_kernel_spmd(nc, in_maps, core_ids=list(range(8)),
                                          trace=_trace)
    if _trace:
        _CACHED["last_exec_time_ns"] = res.exec_time_ns
        _CACHED["last_trace"] = res.instructions_and_trace
    bs = x.shape[0]
    out = np.zeros((bs, SEQ, DIM), dtype=np.float32)
    for core in range(8):
        out[core // 4] += res.results[core]["out"]
    return out


# revision 31
# speedup vs baseline: 1.1970x; 1.1970x over previous
"""Causal multi-head attention with RoPE on 8 TRN2 NeuronCores.

Problem: BS=2, SEQ=2048, DIM=2048, NH=16, HD=128 (fp32 in/out).
Sharding: core = b*4 + g  (b = batch, g = head-group of 4 heads).
Each core computes q/k/v for its 4 heads from its batch's x, applies RoPE,
causal attention, and a partial output projection through its 512-row slice
of wo. The host sums the 4 per-group partials per batch.

v4 (bf16 operands):
  Phase V first (sb-major, needs only 1/4 of x to start) while the rest of
  x and the K/Q weights stream in; then K+Q m-major (weight tile fixed,
  4 seq-blocks inner -> lhsT reuse). Q is scaled by 1/sqrt(HD) during the
  PSUM->SBUF copy so K and Q share one trig table. x stays resident in
  SBUF across both passes. Attention processes score tiles in PAIRS
  sharing a 2-bank PSUM tile so one ACT exp covers 1024 columns; diagonal
  blocks trimmed to (512,384,256,128) columns. Rowsum via ones-matmul.
  1/rowsum = ACT Ln + Exp(-x), emitted lazily after the next block's first
  pair so table swaps stay off the PSUM-reuse critical path. wo pass pairs
  dblk matmuls per lhsT and interleaves with the next block's attention.
"""
import math
import numpy as np
from contextlib import ExitStack

import concourse.bass as bass
import concourse.bacc as bacc
import concourse.tile as tile
import concourse.mybir as mybir
from concourse import bass_utils

F32 = mybir.dt.float32
F32R = mybir.dt.float32r
BF16 = mybir.dt.bfloat16
AF = mybir.ActivationFunctionType

SEQ = 2048
DIM = 2048
HD = 128
MG = 512                       # per-core head width (4 heads x 128)
ND = DIM // 128                # 16 d-tiles
PAIR_SWAP = [i ^ 1 for i in range(32)]

MM_DT = BF16                   # matmul operand dtype

_CACHED = {}


def build_nc(seq=SEQ, mm_dt=None):
    mm_dt = mm_dt or MM_DT
    NSB = seq // 512
    NST = seq // 128
    qscale = float(1.0 / math.sqrt(HD))
    nc = bacc.Bacc("TRN2", target_bir_lowering=False, debug=False)

    x_d = nc.dram_tensor("x_t", [NSB, 128, ND, 512], mm_dt, kind="ExternalInput")
    wq_d = nc.dram_tensor("wq_t", [128, ND, 512], mm_dt, kind="ExternalInput")
    wk_d = nc.dram_tensor("wk_t", [128, ND, 512], mm_dt, kind="ExternalInput")
    wv_d = nc.dram_tensor("wv_t", [128, ND, 512], mm_dt, kind="ExternalInput")
    wo_d = nc.dram_tensor("wo_t", [128, 4, DIM], mm_dt, kind="ExternalInput")
    trk_d = nc.dram_tensor("trigk", [128, 2, seq], F32, kind="ExternalInput")
    msk_d = nc.dram_tensor("masks_t", [128, 4, 512], BF16, kind="ExternalInput")
    onc_d = nc.dram_tensor("ones_sq", [128, 128], mm_dt, kind="ExternalInput")
    out_d = nc.dram_tensor("out", [seq, DIM], F32, kind="ExternalOutput")

    with tile.TileContext(nc) as tc, ExitStack() as ctx:
        persist = ctx.enter_context(tc.tile_pool(name="persist", bufs=1))
        ktr = [persist.tile([128, seq], mm_dt, tag=f"ktr{h}", name=f"ktr{h}")
               for h in range(4)]
        v_sb = persist.tile([128, NST, MG], mm_dt, tag="v")
        ones_sq = persist.tile([128, 128], mm_dt, tag="onesq")
        msk = persist.tile([128, 4, 512], BF16, tag="masks")
        qtrp = ctx.enter_context(tc.tile_pool(name="qtrp", bufs=1))
        qtr = [qtrp.tile([128, seq], mm_dt, tag=f"qtr{h}", name=f"qtr{h}")
               for h in range(4)]

        # trig + rope scratch: DVE-consumed; keep open past phase A so the
        # pre-attention pool drain only waits on PE-consumed tiles
        trigp = ctx.enter_context(tc.tile_pool(name="trigp", bufs=1))
        trigk_sb = trigp.tile([128, 2, seq], F32, tag="trigk")
        prawp = ctx.enter_context(tc.tile_pool(name="prawp", bufs=3))
        ropet = ctx.enter_context(tc.tile_pool(name="ropep", bufs=2))

        # long-lived within phases V+A only: x and the K/Q weights (PE-read)
        s1 = ExitStack()
        xpool = s1.enter_context(tc.tile_pool(name="xp", bufs=1))
        xall = xpool.tile([128, NSB, ND, 512], mm_dt, tag="xall")
        paw = s1.enter_context(tc.tile_pool(name="paw", bufs=1))
        wk_sb = paw.tile([128, ND, MG], mm_dt, tag="wk")
        wq_sb = paw.tile([128, ND, MG], mm_dt, tag="wq")

        def rope(psum_t, out_slice, trig_t, scale=1.0):
            """out = p*cos + shuffle(p)*sin; ACT-copy psum->sbuf first so the
            PSUM bank frees without waiting the DVE chain."""
            praw = prawp.tile([128, 512], F32, tag="praw", name="praw")
            if scale == 1.0:
                nc.scalar.copy(praw[:], psum_t[:])
            else:
                nc.scalar.activation(praw[:], psum_t[:], AF.Copy, scale=scale)
            shuf = ropet.tile([128, 512], F32, tag="shuf", name="shuf")
            nc.vector.stream_shuffle(shuf[:], praw[:], PAIR_SWAP)
            t1 = ropet.tile([128, 512], F32, tag="t1", name="t1")
            nc.vector.tensor_mul(t1[:], praw[:], trig_t[:, 0, :])
            nc.vector.tensor_mul(shuf[:], shuf[:], trig_t[:, 1, :])
            nc.vector.tensor_add(out_slice, t1[:], shuf[:])

        def load_w_quarters(dst, dram_ap):
            for q4 in range(4):
                nc.sync.dma_start(dst[:, q4 * 4:(q4 + 1) * 4, :],
                                  dram_ap[:, q4 * 4:(q4 + 1) * 4, :])

        # ---- phase V: V = x^T @ wv, sb-major; x/weights stream in ----
        with tc.tile_pool(name="pbw", bufs=1) as pbw, \
             tc.tile_pool(name="psv", bufs=1, space="PSUM") as psv:
            wv_w = pbw.tile([128, ND, MG], mm_dt, tag="wvw")
            # first V matmul needs only wv quarter 0 + x[sb0, dt0-1]:
            # issue those triggers first so PE starts ~4us in
            nc.sync.dma_start(wv_w[:, 0:4, :], wv_d.ap()[:, 0:4, :])
            nc.sync.dma_start(xall[:, 0, 0:2, :], x_d.ap()[0, :, 0:2, :])
            nc.sync.dma_start(xall[:, 0, 2:8, :], x_d.ap()[0, :, 2:8, :])
            for q4 in range(1, 4):
                nc.sync.dma_start(wv_w[:, q4 * 4:(q4 + 1) * 4, :],
                                  wv_d.ap()[:, q4 * 4:(q4 + 1) * 4, :])
            nc.sync.dma_start(xall[:, 0, 8:16, :], x_d.ap()[0, :, 8:16, :])
            nc.sync.dma_start(ones_sq[:], onc_d.ap())
            nc.sync.dma_start(msk[:], msk_d.ap())
            for sb in range(1, NSB):
                nc.sync.dma_start(xall[:, sb, 0:8, :], x_d.ap()[sb, :, 0:8, :])
                nc.sync.dma_start(xall[:, sb, 8:16, :],
                                  x_d.ap()[sb, :, 8:16, :])
            load_w_quarters(wk_sb, wk_d.ap())
            load_w_quarters(wq_sb, wq_d.ap())
            nc.sync.dma_start(trigk_sb[:], trk_d.ap())
            for sb in range(NSB):
                ps_v = [psv.tile([128, 512], F32, tag=f"psv{st}",
                                 name=f"psv{st}") for st in range(4)]
                for dt in range(ND):
                    for st in range(4):
                        nc.tensor.matmul(
                            ps_v[st][:],
                            xall[:, sb, dt, st * 128:(st + 1) * 128],
                            wv_w[:, dt, :],
                            start=(dt == 0), stop=(dt == ND - 1))
                for st in range(4):
                    nc.scalar.copy(v_sb[:, sb * 4 + st, :], ps_v[st][:])

        # ---- phase A: K + Q, m-major (lhsT weight reused across 4 sb) ----
        with tc.tile_pool(name="psk", bufs=1, space="PSUM") as psk, \
             tc.tile_pool(name="psq", bufs=1, space="PSUM") as psq:
            for g in range(4):
                ps_k = [psk.tile([128, 512], F32, tag=f"psk{sb}",
                                 name=f"psk{sb}") for sb in range(NSB)]
                for dt in range(ND):
                    for sb in range(NSB):
                        nc.tensor.matmul(
                            ps_k[sb][:],
                            wk_sb[:, dt, g * 128:(g + 1) * 128],
                            xall[:, sb, dt, :],
                            start=(dt == 0), stop=(dt == ND - 1))
                for sb in range(NSB):
                    rope(ps_k[sb], ktr[g][:, sb * 512:(sb + 1) * 512],
                         trigk_sb[:, :, sb * 512:(sb + 1) * 512])
                ps_q = [psq.tile([128, 512], F32, tag=f"psq{sb}",
                                 name=f"psq{sb}") for sb in range(NSB)]
                if g < 3:
                    for dt in range(ND):
                        for sb in range(NSB):
                            nc.tensor.matmul(
                                ps_q[sb][:],
                                wq_sb[:, dt, g * 128:(g + 1) * 128],
                                xall[:, sb, dt, :],
                                start=(dt == 0), stop=(dt == ND - 1))
                    for sb in range(NSB):
                        rope(ps_q[sb], qtr[g][:, sb * 512:(sb + 1) * 512],
                             trigk_sb[:, :, sb * 512:(sb + 1) * 512],
                             scale=qscale)
                else:
                    # last group sb-outer: each seq-block's rope fires as its
                    # accumulation stops, so the final rope's DVE chain does
                    # not trail the last matmul by the whole group
                    for sb in range(NSB):
                        for dt in range(ND):
                            nc.tensor.matmul(
                                ps_q[sb][:],
                                wq_sb[:, dt, g * 128:(g + 1) * 128],
                                xall[:, sb, dt, :],
                                start=(dt == 0), stop=(dt == ND - 1))
                        rope(ps_q[sb], qtr[g][:, sb * 512:(sb + 1) * 512],
                             trigk_sb[:, :, sb * 512:(sb + 1) * 512],
                             scale=qscale)

        s1.close()

        # ---- phase 2: attention + wo ----
        with tc.tile_pool(name="p2w", bufs=1) as p2wp, \
             tc.tile_pool(name="ep", bufs=3) as ep, \
             tc.tile_pool(name="etmp", bufs=2) as etmp, \
             tc.tile_pool(name="otn", bufs=2) as otn, \
             tc.tile_pool(name="bcp", bufs=2) as bcp, \
             tc.tile_pool(name="wout", bufs=3) as wout, \
             tc.tile_pool(name="ps_s", bufs=2, space="PSUM") as ps_s, \
             tc.tile_pool(name="ps_o", bufs=1, space="PSUM") as ps_o, \
             tc.tile_pool(name="ps_r", bufs=1, space="PSUM") as ps_r, \
             tc.tile_pool(name="ps_w", bufs=1, space="PSUM") as ps_w:
            wo_sb = p2wp.tile([128, 4, DIM], mm_dt, tag="wo")
            for c in range(4):
                nc.sync.dma_start(wo_sb[:, c, :], wo_d.ap()[:, c, :])

            def wo_block(outn_t, ibp, it, tail=False):
                i0p = ibp * 512
                for half in range(2):
                    pw = [ps_w.tile([128, 512], F32, tag=f"w{dh}",
                                    name=f"w{dh}") for dh in range(2)]
                    for c in range(4):
                        for dh in range(2):
                            dblk = half * 2 + dh
                            nc.tensor.matmul(
                                pw[dh][:],
                                outn_t[:, c, it * 128:(it + 1) * 128],
                                wo_sb[:, c, dblk * 512:(dblk + 1) * 512],
                                start=(c == 0), stop=(c == 3))
                    for dh in range(2):
                        dblk = half * 2 + dh
                        ow = wout.tile([128, 512], F32, tag="ow", name="ow")
                        if tail:
                            # ACT is idle in the tail; DVE is the gate there
                            nc.scalar.copy(ow[:], pw[dh][:])
                        else:
                            nc.vector.tensor_copy(ow[:], pw[dh][:])
                        nc.sync.dma_start(
                            out_d.ap()[i0p + it * 128:i0p + (it + 1) * 128,
                                       dblk * 512:(dblk + 1) * 512], ow[:])

            prev_outn = [None]
            for ib in range(NSB):
                i0 = ib * 512
                nj = 4 * ib + 4
                outn = otn.tile([128, 4, 512], mm_dt, tag="outn", name="outn")
                for h in range(4):
                    po = ps_o.tile([128, 512], F32, tag="pv", name="pv")
                    prbc = ps_r.tile([128, 512], F32, tag="rs", name="rs")
                    for tp in range(nj // 2):
                        specs = []
                        for q in range(2):
                            tj = 2 * tp + q
                            r = tj - 4 * ib
                            i_lo = 128 * r if r > 0 else 0
                            specs.append((tj, r, i_lo, 512 - i_lo))
                        pscr = ps_s.tile([128, 2, 512], F32, tag="sc", name="sc")
                        for q, (tj, r, i_lo, nw) in enumerate(specs):
                            nc.tensor.matmul(
                                pscr[:, q, 0:nw],
                                ktr[h][:, tj * 128:(tj + 1) * 128],
                                qtr[h][:, i0 + i_lo:i0 + i_lo + nw],
                                start=True, stop=True)
                        e2 = ep.tile([128, 2, 512], mm_dt, tag="e", name="e")
                        if specs[0][1] >= 0 or specs[1][1] >= 0:
                            for q, (tj, r, i_lo, nw) in enumerate(specs):
                                et = etmp.tile([128, 512], mm_dt, tag="etmp",
                                               name="et")
                                nc.scalar.activation(et[:, 0:nw],
                                                     pscr[:, q, 0:nw], AF.Exp)
                                nc.vector.tensor_mul(e2[:, q, 0:nw],
                                                     et[:, 0:nw],
                                                     msk[:, r, i_lo:512])
                        else:
                            nc.scalar.activation(e2[:, 0:2, :],
                                                 pscr[:, 0:2, :], AF.Exp)
                        for q, (tj, r, i_lo, nw) in enumerate(specs):
                            nc.tensor.matmul(
                                po[:, i_lo:i_lo + nw],
                                v_sb[:, tj, h * 128:(h + 1) * 128],
                                e2[:, q, 0:nw],
                                start=(tj == 0), stop=(tj == nj - 1))
                            nc.tensor.matmul(
                                prbc[:, i_lo:i_lo + nw], ones_sq[:],
                                e2[:, q, 0:nw],
                                start=(tj == 0), stop=(tj == nj - 1))
                    nc.vector.tensor_copy(outn[:, h, :], po[:])
                    # 1/rowsum on DVE straight from PSUM (no ACT table swaps)
                    rbc = bcp.tile([128, 512], F32, tag="rbc", name="rbc")
                    nc.vector.reciprocal_approx_fast(rbc[:], prbc[:])
                    rbh = bcp.tile([128, 512], mm_dt, tag="rbh", name="rbh")
                    nc.vector.tensor_copy(rbh[:], rbc[:])
                    nc.vector.tensor_mul(outn[:, h, :], outn[:, h, :], rbh[:])
                    if prev_outn[0] is not None:
                        wo_block(prev_outn[0], ib - 1, h)
                prev_outn[0] = outn

            for it in range(4):
                wo_block(prev_outn[0], NSB - 1, it, tail=True)

    nc.compile()
    return nc


def _host_prep(x, freqs_cos, freqs_sin, wq, wk, wv, wo, mm_dt=None, seq=SEQ):
    """Build the 8 per-core input maps with pre-tiled layouts."""
    mm_dt = mm_dt or MM_DT
    npdt = mybir.dt.np(mm_dt)
    bs = x.shape[0]
    NSB = seq // 512

    cos_e = np.repeat(np.asarray(freqs_cos).T, 2, axis=0).astype(np.float32)
    sin_raw = np.repeat(np.asarray(freqs_sin).T, 2, axis=0).astype(np.float32)
    sin_e = sin_raw.copy()
    sin_e[0::2] = -sin_raw[0::2]      # out[2i] = q[2i]cos - q[2i+1]sin
    trigk = np.ascontiguousarray(np.stack([cos_e, sin_e], axis=1))

    jr = np.arange(128)[:, None]
    ir = np.arange(512)[None, :]
    masks = np.zeros((4, 128, 512), dtype=np.float32)
    for r in range(4):
        masks[r] = (128 * r + jr <= ir).astype(np.float32)
    masks_t = np.ascontiguousarray(masks.transpose(1, 0, 2)).astype(mybir.dt.np(BF16))

    ones_sq = np.ones((128, 128), npdt)

    def wtile(w):  # [DIM, 512] -> [128, 16, 512]
        return np.ascontiguousarray(
            np.asarray(w).reshape(ND, 128, MG).transpose(1, 0, 2)).astype(npdt)

    x_t = []
    for b in range(bs):
        xt = np.asarray(x[b]).reshape(NSB, 512, ND, 128).transpose(0, 3, 2, 1)
        x_t.append(np.ascontiguousarray(xt).astype(npdt))

    in_maps = []
    for core in range(8):
        b, g = divmod(core, 4)
        b = min(b, bs - 1)
        wo_g = np.asarray(wo)[g * MG:(g + 1) * MG, :]
        in_maps.append({
            "x_t": x_t[b],
            "wq_t": wtile(np.asarray(wq)[:, g * MG:(g + 1) * MG]),
            "wk_t": wtile(np.asarray(wk)[:, g * MG:(g + 1) * MG]),
            "wv_t": wtile(np.asarray(wv)[:, g * MG:(g + 1) * MG]),
            "wo_t": np.ascontiguousarray(
                wo_g.reshape(4, 128, DIM).transpose(1, 0, 2)).astype(npdt),
            "trigk": trigk,
            "masks_t": masks_t, "ones_sq": ones_sq,
        })
    return in_maps


def kernel(x, freqs_cos, freqs_sin, mask, wq, wk, wv, wo, _trace=False):
    x = np.asarray(x, dtype=np.float32)
    in_maps = _host_prep(x, np.asarray(freqs_cos), np.asarray(freqs_sin),
                         np.asarray(wq), np.asarray(wk), np.asarray(wv),
                         np.asarray(wo))
    if "nc" not in _CACHED:
        _CACHED["nc"] = build_nc()
    nc = _CACHED["nc"]
    res = bass_utils.run_bass_kernel_spmd(nc, in_maps, core_ids=list(range(8)),
                                          trace=_trace)
    if _trace:
        _CACHED["last_exec_time_ns"] = res.exec_time_ns
        _CACHED["last_trace"] = res.instructions_and_trace
    bs = x.shape[0]
    out = np.zeros((bs, SEQ, DIM), dtype=np.float32)
    for core in range(8):
        out[core // 4] += res.results[core]["out"]
    return out
